# revision 11
# baseline (speedup 1.0000x reference)
"""Expert-parallel MoE (CompoundMoELayer) kernel for 8 Trainium2 NeuronCores.

Strategy (per the expert-parallelism sharding hint):
  - Router runs once on the host (it is <0.2% of the FLOPs and is the
    sharding decision itself): logits -> top-4 -> softmax combine weights.
  - The 32 experts are sharded 4-per-core.  For each expert the tokens routed
    to it are gathered into a padded [D, C] transposed batch (column-major
    tokens) so the device only does dense GEMMs.
  - Each core runs, per owned expert:  H_T = W1^T @ XeT  (PSUM, +b1, exact
    GELU) ; Y = G @ W2 ; Y *= combine_weight.  Y is returned per expert slot.
  - Host scatter-adds the weighted per-expert outputs back to token order
    (the "combine" of the all-to-all) and adds the (combine @ b2) term.
  - The aux scalar (load-balance loss + entropy regularizer) is a tiny
    reduction over the router logits, computed on host in float64.

The device program is an SPMD Bass/Tile kernel launched on all 8 cores via
run_bass_kernel_spmd; per-core inputs differ only in data (expert slices).
"""

import os
import sys

import numpy as np

for _p in ("/opt/trn_rl_repo", "/root/.axon_site/_ro/trn_rl_repo"):
    if os.path.isdir(_p) and _p not in sys.path:
        sys.path.append(_p)

import concourse.bass as bass
import concourse.mybir as mybir
import concourse.tile as tile
from concourse.bass_utils import run_bass_kernel_spmd
from concourse.vector_clock import ScopedClock

# ----------------------------------------------------------------------------
# This container's walrus build only accepts a single sync-wait on a Drain
# instruction; TileContext's tail drain attaches one wait per live DMA queue.
# Split the waits across multiple Drain instructions (same semantics: they
# run back-to-back on the sync engine before the all-engine barrier).
# ----------------------------------------------------------------------------


def _split_multi_waits(nc):
    """Hoist extra sync-waits into preceding same-engine nops (one wait per
    instruction), for every instruction in every basic block."""
    for bb in nc.m.functions[0].blocks:
        insts = bb.instructions
        i = 0
        while i < len(insts):
            inst = insts[i]
            si = inst.sync_info
            if si is not None and si.on_wait is not None and len(si.on_wait) > 1:
                waits = list(si.on_wait)
                del si.on_wait[: len(waits) - 1]  # keep only the last wait
                eng = inst.engine
                for w in waits[:-1]:
                    nop = mybir.InstNoOp(
                        name=nc.get_next_instruction_name(), ins=[], outs=[]
                    )
                    nop.engine = eng
                    nop.sync_info = mybir.SyncInfo(on_wait=[w], on_update=[])
                    nc.register_instruction(nop, overwrite=True)
                    insts.insert(i, nop)
                    i += 1
            i += 1


def _drain_and_barrier_split(self, tick_clock, wait_clock):
    nc = self.nc
    drain_inst = nc.sync.drain()
    wait_clock.add_sem_waits(
        drain_inst.ins, ScopedClock({None: tick_clock.global_clock})
    )
    si = drain_inst.ins.sync_info
    if si is not None and si.on_wait is not None and len(si.on_wait) > 1:
        waits = list(si.on_wait)
        del si.on_wait[1:]
        for w in waits[1:]:
            extra = nc.sync.drain()
            esi = extra.ins.sync_info
            if esi is None:
                extra.ins.sync_info = mybir.SyncInfo(on_wait=[w], on_update=[])
            else:
                if esi.on_wait is None:
                    esi.on_wait = []
                esi.on_wait.append(w)

    _split_multi_waits(nc)

    nc.all_engine_barrier()
    assert self.sems is not None
    popped = nc._tile_sem_poison_stack.pop()
    assert popped is self._sem_poison
    nc.clear_and_free_semaphores(list(self.sems.allocated().values()))
    nc.all_engine_barrier()


tile.TileContext._drain_and_barrier = _drain_and_barrier_split

# ----------------------------------------------------------------------------
# Problem constants (hardcoded per the harness contract).
# ----------------------------------------------------------------------------
N_CORES = 8
E = 32
D = 1024
F = 2048
N_TOK = 1024
E_LOC = E // N_CORES  # experts per core

_F32 = mybir.dt.float32

_PROG_CACHE: dict = {}
LAST_RESULTS = None  # BassKernelResults of the most recent launch (for test.py)


def _ensure_ntff_hook():
    """Profiling support: this agent image ships no ``antenv.axon_hooks``,
    so ``run_bass_kernel_spmd(trace=True)`` under axon degrades to no
    timing.  Provide the module + the ctypes NTFF hook (mirror of
    trn_boot._ntff_profile_via_ctypes), and neuter the fish upload."""
    import contextlib
    import ctypes
    import types

    name = "antenv.axon_hooks"
    if name not in sys.modules:
        mod = types.ModuleType(name)
        mod._hook = None

        def set_axon_ntff_profile_hook(h):
            mod._hook = h

        def get_axon_ntff_profile_hook():
            return mod._hook

        mod.set_axon_ntff_profile_hook = set_axon_ntff_profile_hook
        mod.get_axon_ntff_profile_hook = get_axon_ntff_profile_hook
        sys.modules[name] = mod

        so_path = "/opt/axon/libaxon_pjrt.so"
        try:
            lib = ctypes.CDLL(so_path)
            lib.axon_start_nrt_profile.argtypes = [
                ctypes.POINTER(ctypes.c_int64),
                ctypes.c_size_t,
            ]
            lib.axon_start_nrt_profile.restype = ctypes.c_int64
            lib.axon_stop_nrt_profile.argtypes = [ctypes.c_char_p]
            lib.axon_stop_nrt_profile.restype = ctypes.c_int64

            @contextlib.contextmanager
            def _hook(output_dir, device_ids):
                import jax

                jax.devices()
                if device_ids:
                    ids = (ctypes.c_int64 * len(device_ids))(*device_ids)
                    rc = lib.axon_start_nrt_profile(ids, len(device_ids))
                else:
                    rc = lib.axon_start_nrt_profile(None, 0)
                if rc != 0:
                    raise RuntimeError(f"axon_start_nrt_profile rc={rc}")
                try:
                    yield
                finally:
                    n = lib.axon_stop_nrt_profile(str(output_dir).encode())
                    print(f"profile: {n} file(s) written to {output_dir}")

            mod.set_axon_ntff_profile_hook(_hook)
        except (OSError, AttributeError):
            pass

    import concourse.bass_utils as _bu

    _bu.upload_artifacts = lambda tmpdir: str(tmpdir)


def _build_program(C: int, dt_mm):
    """Per-core SPMD program: E_LOC experts, capacity C tokens each."""
    KT = D // 128  # contraction tiles for GEMM1
    FT = F // 128  # FFN tiles (partition dim of H_T)
    CT = C // 128  # token tiles
    JT = D // 512  # output-column tiles for GEMM2

    nc = bass.Bass()
    xT = nc.dram_tensor("xT", [E_LOC, D, C], dt_mm, kind="ExternalInput")
    w1 = nc.dram_tensor("w1", [E_LOC, D, F], dt_mm, kind="ExternalInput")
    b1v = nc.dram_tensor("b1v", [E_LOC, 128, FT], _F32, kind="ExternalInput")
    w2 = nc.dram_tensor("w2", [E_LOC, F, D], dt_mm, kind="ExternalInput")
    wt = nc.dram_tensor("wt", [E_LOC, 128, CT], _F32, kind="ExternalInput")
    y = nc.dram_tensor("y", [E_LOC, C, D], _F32, kind="ExternalOutput")

    gelu = mybir.ActivationFunctionType.Gelu

    with tile.TileContext(nc) as tc:
        with (
            tc.tile_pool(name="xp", bufs=2) as xp,
            tc.tile_pool(name="w1p", bufs=3) as w1p,
            tc.tile_pool(name="gp", bufs=2) as gp,
            tc.tile_pool(name="w2p", bufs=3) as w2p,
            tc.tile_pool(name="yp", bufs=4) as yp,
            tc.tile_pool(name="cst", bufs=2) as cst,
            tc.tile_pool(name="ps1", bufs=2, space="PSUM") as ps1,
            tc.tile_pool(name="ps2", bufs=2, space="PSUM") as ps2,
        ):
            for e in range(E_LOC):
                # gathered tokens, transposed: partition = d % 128, free = (k-tile, token)
                xsb = xp.tile([128, KT, C], dt_mm, tag="xsb")
                nc.sync.dma_start(
                    xsb[:], xT[e].rearrange("(kk p) c -> p kk c", p=128)
                )
                b1sb = cst.tile([128, FT], _F32, tag="b1sb")
                nc.sync.dma_start(b1sb[:], b1v[e])
                wtsb = cst.tile([128, CT], _F32, tag="wtsb")
                nc.sync.dma_start(wtsb[:], wt[e])

                # G^T = gelu(W1^T @ Xe^T + b1), laid out [FFN-part, token]
                gsb = gp.tile([128, FT, C], dt_mm, tag="gsb")
                for f in range(FT):
                    w1sb = w1p.tile([128, KT, 128], dt_mm, tag="w1sb")
                    nc.sync.dma_start(
                        w1sb[:],
                        w1[e, :, f * 128 : (f + 1) * 128].rearrange(
                            "(kk p) m -> p kk m", p=128
                        ),
                    )
                    ph = ps1.tile([128, C], _F32, tag="ph")
                    for kk in range(KT):
                        nc.tensor.matmul(
                            ph[:],
                            w1sb[:, kk, :],
                            xsb[:, kk, :],
                            start=(kk == 0),
                            stop=(kk == KT - 1),
                        )
                    nc.scalar.activation(
                        gsb[:, f, :],
                        ph[:],
                        gelu,
                        bias=b1sb[:, f : f + 1],
                    )

                # Y = (G @ W2) * combine_weight, tiled [token-tile, 512-col]
                for j in range(JT):
                    phys = [
                        ps2.tile([128, 512], _F32, tag=f"py{c}", name=f"py{c}")
                        for c in range(CT)
                    ]
                    for fk in range(FT):
                        w2sb = w2p.tile([128, 512], dt_mm, tag="w2sb")
                        nc.sync.dma_start(
                            w2sb[:],
                            w2[e, fk * 128 : (fk + 1) * 128, j * 512 : (j + 1) * 512],
                        )
                        for c in range(CT):
                            nc.tensor.matmul(
                                phys[c][:],
                                gsb[:, fk, c * 128 : c * 128 + 128],
                                w2sb[:],
                                start=(fk == 0),
                                stop=(fk == FT - 1),
                            )
                    for c in range(CT):
                        ysb = yp.tile([128, 512], _F32, tag="ysb")
                        nc.vector.tensor_scalar_mul(
                            ysb[:], phys[c][:], wtsb[:, c : c + 1]
                        )
                        nc.sync.dma_start(
                            y[e, c * 128 : (c + 1) * 128, j * 512 : (j + 1) * 512],
                            ysb[:],
                        )
    return nc


def _route(flat, gate_w, gate_b, k):
    """Host router in float64: returns (top_idx [N,k], weights [N,k], logits)."""
    logits = flat.astype(np.float64) @ gate_w.astype(np.float64) + gate_b.astype(
        np.float64
    )
    # stable argsort matches jax.lax.top_k tie-breaking (lower index first)
    top_idx = np.argsort(-logits, axis=1, kind="stable")[:, :k]
    top_vals = np.take_along_axis(logits, top_idx, axis=1)
    w = np.exp(top_vals - top_vals.max(axis=1, keepdims=True))
    w /= w.sum(axis=1, keepdims=True)
    return top_idx, w, logits


def kernel(**inputs) -> tuple:
    global LAST_RESULTS
    x = np.asarray(inputs["x"])
    gate_w = np.asarray(inputs["gate_w"], np.float32)
    gate_b = np.asarray(inputs["gate_b"], np.float32)
    w1 = np.asarray(inputs["w1"], np.float32)
    b1 = np.asarray(inputs["b1"], np.float32)
    w2 = np.asarray(inputs["w2"], np.float32)
    b2 = np.asarray(inputs["b2"], np.float32)
    load_ema = np.asarray(inputs["load_ema"], np.float32)
    k = int(np.asarray(inputs["top_k"]))

    B, S, d = x.shape
    n_tok = B * S
    flat = np.ascontiguousarray(x.reshape(n_tok, d), dtype=np.float32)
    n_experts = gate_w.shape[1]

    # ---- host router (this is the sharding decision) ----
    top_idx, weights, logits = _route(flat, gate_w, gate_b, k)

    idx_by_e = []
    wt_by_e = []
    for e in range(n_experts):
        tok, slot = np.nonzero(top_idx == e)
        idx_by_e.append(tok)
        wt_by_e.append(weights[tok, slot].astype(np.float32))
    counts = np.array([len(ix) for ix in idx_by_e])

    C = max(256, 128 * int(np.ceil(counts.max() / 128)))
    dt_mm = _F32

    key = (C, str(dt_mm), E_LOC)
    if key not in _PROG_CACHE:
        _PROG_CACHE[key] = _build_program(C, dt_mm)
    nc = _PROG_CACHE[key]

    # ---- build per-core inputs (expert slices + gathered token batches) ----
    assign = [list(range(c * E_LOC, (c + 1) * E_LOC)) for c in range(N_CORES)]
    flatT = np.ascontiguousarray(flat.T)  # [D, N]
    FT = F // 128
    CT = C // 128

    in_maps = []
    for core in range(N_CORES):
        exps = assign[core]
        xT_c = np.zeros((E_LOC, D, C), np.float32)
        wt_c = np.zeros((E_LOC, C), np.float32)
        b1_c = np.zeros((E_LOC, 128, FT), np.float32)
        for s, e in enumerate(exps):
            cnt = counts[e]
            xT_c[s, :, :cnt] = flatT[:, idx_by_e[e]]
            wt_c[s, :cnt] = wt_by_e[e]
            b1_c[s] = b1[e].reshape(FT, 128).T
        in_maps.append(
            {
                "xT": xT_c,
                "w1": np.ascontiguousarray(w1[exps]),
                "b1v": b1_c,
                "w2": np.ascontiguousarray(w2[exps]),
                "wt": np.ascontiguousarray(
                    wt_c.reshape(E_LOC, CT, 128).transpose(0, 2, 1)
                ),
            }
        )

    trace = bool(int(os.environ.get("KERNEL_TRACE", "0")))
    if trace:
        _ensure_ntff_hook()
    res = run_bass_kernel_spmd(nc, in_maps, list(range(N_CORES)), trace=trace)
    LAST_RESULTS = res

    # ---- combine (the "all-to-all combine" done host-side) ----
    out = np.zeros((n_tok, d), np.float64)
    for core in range(N_CORES):
        y_c = res.results[core]["y"]
        for s, e in enumerate(assign[core]):
            cnt = counts[e]
            out[idx_by_e[e]] += y_c[s, :cnt].astype(np.float64)
            if b2[e].any():
                out[idx_by_e[e]] += (
                    wt_by_e[e][:, None].astype(np.float64) * b2[e][None, :]
                )

    # ---- aux loss (host, float64) ----
    probs = np.exp(logits - logits.max(axis=1, keepdims=True))
    probs /= probs.sum(axis=1, keepdims=True)
    frac = counts / n_tok / k
    base_aux = n_experts * np.sum(frac * probs.mean(axis=0))
    le = load_ema.astype(np.float64)
    lp = le / (le.sum() + 1e-8)
    entropy = -np.sum(lp * np.log(lp + 1e-8))
    reg = np.log(float(n_experts)) - entropy
    aux = np.float32(base_aux + 0.001 * reg)

    return out.astype(np.float32).reshape(B, S, d), aux


# revision 17
# speedup vs baseline: 1.9574x; 1.9574x over previous
"""Expert-parallel MoE (CompoundMoELayer) kernel for 8 Trainium2 NeuronCores.

Strategy (per the expert-parallelism sharding hint):
  - Router runs once on the host (it is <0.2% of the FLOPs and is the
    sharding decision itself): logits -> top-4 -> softmax combine weights.
  - The 32 experts are sharded 4-per-core.  For each expert the tokens routed
    to it are gathered into a padded [D, C] transposed batch (column-major
    tokens) so the device only does dense GEMMs.
  - Each core runs, per owned expert:  H_T = W1^T @ XeT  (PSUM, +b1, exact
    GELU) ; Y = G @ W2 ; Y *= combine_weight.  Y is returned per expert slot.
  - Host scatter-adds the weighted per-expert outputs back to token order
    (the "combine" of the all-to-all) and adds the (combine @ b2) term.
  - The aux scalar (load-balance loss + entropy regularizer) is a tiny
    reduction over the router logits, computed on host in float64.

The device program is an SPMD Bass/Tile kernel launched on all 8 cores via
run_bass_kernel_spmd; per-core inputs differ only in data (expert slices).
"""

import os
import sys

import numpy as np

for _p in ("/opt/trn_rl_repo", "/root/.axon_site/_ro/trn_rl_repo"):
    if os.path.isdir(_p) and _p not in sys.path:
        sys.path.append(_p)

import concourse.bass as bass
import concourse.mybir as mybir
import concourse.tile as tile
from concourse.bass_utils import run_bass_kernel_spmd
from concourse.vector_clock import ScopedClock

# ----------------------------------------------------------------------------
# This container's walrus build only accepts a single sync-wait on a Drain
# instruction; TileContext's tail drain attaches one wait per live DMA queue.
# Split the waits across multiple Drain instructions (same semantics: they
# run back-to-back on the sync engine before the all-engine barrier).
# ----------------------------------------------------------------------------


def _split_multi_waits(nc):
    """Hoist extra sync-waits into preceding same-engine nops (one wait per
    instruction), for every instruction in every basic block."""
    for bb in nc.m.functions[0].blocks:
        insts = bb.instructions
        i = 0
        while i < len(insts):
            inst = insts[i]
            si = inst.sync_info
            if si is not None and si.on_wait is not None and len(si.on_wait) > 1:
                waits = list(si.on_wait)
                del si.on_wait[: len(waits) - 1]  # keep only the last wait
                eng = inst.engine
                for w in waits[:-1]:
                    nop = mybir.InstNoOp(
                        name=nc.get_next_instruction_name(), ins=[], outs=[]
                    )
                    nop.engine = eng
                    nop.sync_info = mybir.SyncInfo(on_wait=[w], on_update=[])
                    nc.register_instruction(nop, overwrite=True)
                    insts.insert(i, nop)
                    i += 1
            i += 1


def _drain_and_barrier_split(self, tick_clock, wait_clock):
    nc = self.nc
    drain_inst = nc.sync.drain()
    wait_clock.add_sem_waits(
        drain_inst.ins, ScopedClock({None: tick_clock.global_clock})
    )
    si = drain_inst.ins.sync_info
    if si is not None and si.on_wait is not None and len(si.on_wait) > 1:
        waits = list(si.on_wait)
        del si.on_wait[1:]
        for w in waits[1:]:
            extra = nc.sync.drain()
            esi = extra.ins.sync_info
            if esi is None:
                extra.ins.sync_info = mybir.SyncInfo(on_wait=[w], on_update=[])
            else:
                if esi.on_wait is None:
                    esi.on_wait = []
                esi.on_wait.append(w)

    _split_multi_waits(nc)

    nc.all_engine_barrier()
    assert self.sems is not None
    popped = nc._tile_sem_poison_stack.pop()
    assert popped is self._sem_poison
    nc.clear_and_free_semaphores(list(self.sems.allocated().values()))
    nc.all_engine_barrier()


tile.TileContext._drain_and_barrier = _drain_and_barrier_split

# ----------------------------------------------------------------------------
# Problem constants (hardcoded per the harness contract).
# ----------------------------------------------------------------------------
N_CORES = 8
E = 32
D = 1024
F = 2048
N_TOK = 1024
E_LOC = E // N_CORES  # experts per core

_F32 = mybir.dt.float32

_PROG_CACHE: dict = {}
LAST_RESULTS = None  # BassKernelResults of the most recent launch (for test.py)


def _ensure_ntff_hook():
    """Profiling support: this agent image ships no ``antenv.axon_hooks``,
    so ``run_bass_kernel_spmd(trace=True)`` under axon degrades to no
    timing.  Provide the module + the ctypes NTFF hook (mirror of
    trn_boot._ntff_profile_via_ctypes), and neuter the fish upload."""
    import contextlib
    import ctypes
    import types

    name = "antenv.axon_hooks"
    if name not in sys.modules:
        mod = types.ModuleType(name)
        mod._hook = None

        def set_axon_ntff_profile_hook(h):
            mod._hook = h

        def get_axon_ntff_profile_hook():
            return mod._hook

        mod.set_axon_ntff_profile_hook = set_axon_ntff_profile_hook
        mod.get_axon_ntff_profile_hook = get_axon_ntff_profile_hook
        sys.modules[name] = mod

        so_path = "/opt/axon/libaxon_pjrt.so"
        try:
            lib = ctypes.CDLL(so_path)
            lib.axon_start_nrt_profile.argtypes = [
                ctypes.POINTER(ctypes.c_int64),
                ctypes.c_size_t,
            ]
            lib.axon_start_nrt_profile.restype = ctypes.c_int64
            lib.axon_stop_nrt_profile.argtypes = [ctypes.c_char_p]
            lib.axon_stop_nrt_profile.restype = ctypes.c_int64

            @contextlib.contextmanager
            def _hook(output_dir, device_ids):
                import jax

                jax.devices()
                if device_ids:
                    ids = (ctypes.c_int64 * len(device_ids))(*device_ids)
                    rc = lib.axon_start_nrt_profile(ids, len(device_ids))
                else:
                    rc = lib.axon_start_nrt_profile(None, 0)
                if rc != 0:
                    raise RuntimeError(f"axon_start_nrt_profile rc={rc}")
                try:
                    yield
                finally:
                    n = lib.axon_stop_nrt_profile(str(output_dir).encode())
                    print(f"profile: {n} file(s) written to {output_dir}")

            mod.set_axon_ntff_profile_hook(_hook)
        except (OSError, AttributeError):
            pass

    import concourse.bass_utils as _bu

    _bu.upload_artifacts = lambda tmpdir: str(tmpdir)


def _build_program(C: int, variant: str):
    """Per-core SPMD program: E_LOC experts, capacity C tokens each.

    variant: "f32" (exact, quarter-rate PE), "f32r" (fp32 storage,
    full-rate single-pass PE), "bf16" (bf16 storage + full-rate PE).
    """
    KT = D // 128  # contraction tiles for GEMM1
    FT = F // 128  # FFN tiles (partition dim of H_T)
    CT = C // 128  # token tiles
    JT = D // 512  # output-column tiles for GEMM2

    dt_mm = {
        "bf16": mybir.dt.bfloat16,
        "f32r": mybir.dt.float32r,
        "f32": _F32,
    }[variant]

    def mm(ap):
        return ap

    nc = bass.Bass()
    xT = nc.dram_tensor("xT", [E_LOC, D, C], dt_mm, kind="ExternalInput")
    w1 = nc.dram_tensor("w1", [E_LOC, D, F], dt_mm, kind="ExternalInput")
    b1v = nc.dram_tensor("b1v", [E_LOC, 128, FT], _F32, kind="ExternalInput")
    w2 = nc.dram_tensor("w2", [E_LOC, F, D], dt_mm, kind="ExternalInput")
    wt = nc.dram_tensor("wt", [E_LOC, 128, CT], _F32, kind="ExternalInput")
    y = nc.dram_tensor("y", [E_LOC, C, D], _F32, kind="ExternalOutput")

    gelu = mybir.ActivationFunctionType.Gelu

    with tile.TileContext(nc) as tc:
        with (
            tc.tile_pool(name="xp", bufs=2) as xp,
            tc.tile_pool(name="w1p", bufs=3) as w1p,
            tc.tile_pool(name="gp", bufs=2) as gp,
            tc.tile_pool(name="w2p", bufs=3) as w2p,
            tc.tile_pool(name="yp", bufs=4) as yp,
            tc.tile_pool(name="cst", bufs=2) as cst,
            tc.tile_pool(name="ps1", bufs=2, space="PSUM") as ps1,
            tc.tile_pool(name="ps2", bufs=2, space="PSUM") as ps2,
        ):
            for e in range(E_LOC):
                # gathered tokens, transposed: partition = d % 128, free = (k-tile, token)
                xsb = xp.tile([128, KT, C], dt_mm, tag="xsb")
                nc.sync.dma_start(
                    xsb[:], xT[e].rearrange("(kk p) c -> p kk c", p=128)
                )
                b1sb = cst.tile([128, FT], _F32, tag="b1sb")
                nc.sync.dma_start(b1sb[:], b1v[e])
                wtsb = cst.tile([128, CT], _F32, tag="wtsb")
                nc.sync.dma_start(wtsb[:], wt[e])

                # G^T = gelu(W1^T @ Xe^T + b1), laid out [FFN-part, token]
                gsb = gp.tile([128, FT, C], dt_mm, tag="gsb")
                for f in range(FT):
                    w1sb = w1p.tile([128, KT, 128], dt_mm, tag="w1sb")
                    nc.sync.dma_start(
                        w1sb[:],
                        w1[e, :, f * 128 : (f + 1) * 128].rearrange(
                            "(kk p) m -> p kk m", p=128
                        ),
                    )
                    ph = ps1.tile([128, C], _F32, tag="ph")
                    for kk in range(KT):
                        nc.tensor.matmul(
                            ph[:],
                            mm(w1sb[:, kk, :]),
                            mm(xsb[:, kk, :]),
                            start=(kk == 0),
                            stop=(kk == KT - 1),
                        )
                    nc.scalar.activation(
                        gsb[:, f, :],
                        ph[:],
                        gelu,
                        bias=b1sb[:, f : f + 1],
                    )

                # Y = (G @ W2) * combine_weight, tiled [token-tile, 512-col]
                for j in range(JT):
                    phys = [
                        ps2.tile([128, 512], _F32, tag=f"py{c}", name=f"py{c}")
                        for c in range(CT)
                    ]
                    for fk in range(FT):
                        w2sb = w2p.tile([128, 512], dt_mm, tag="w2sb")
                        nc.sync.dma_start(
                            w2sb[:],
                            w2[e, fk * 128 : (fk + 1) * 128, j * 512 : (j + 1) * 512],
                        )
                        for c in range(CT):
                            nc.tensor.matmul(
                                phys[c][:],
                                mm(gsb[:, fk, c * 128 : c * 128 + 128]),
                                mm(w2sb[:]),
                                start=(fk == 0),
                                stop=(fk == FT - 1),
                            )
                    for c in range(CT):
                        ysb = yp.tile([128, 512], _F32, tag="ysb")
                        nc.vector.tensor_scalar_mul(
                            ysb[:], phys[c][:], wtsb[:, c : c + 1]
                        )
                        nc.sync.dma_start(
                            y[e, c * 128 : (c + 1) * 128, j * 512 : (j + 1) * 512],
                            ysb[:],
                        )
    return nc


def _route(flat, gate_w, gate_b, k):
    """Host router in float64: returns (top_idx [N,k], weights [N,k], logits)."""
    logits = flat.astype(np.float64) @ gate_w.astype(np.float64) + gate_b.astype(
        np.float64
    )
    # stable argsort matches jax.lax.top_k tie-breaking (lower index first)
    top_idx = np.argsort(-logits, axis=1, kind="stable")[:, :k]
    top_vals = np.take_along_axis(logits, top_idx, axis=1)
    w = np.exp(top_vals - top_vals.max(axis=1, keepdims=True))
    w /= w.sum(axis=1, keepdims=True)
    return top_idx, w, logits


def kernel(**inputs) -> tuple:
    global LAST_RESULTS
    x = np.asarray(inputs["x"])
    gate_w = np.asarray(inputs["gate_w"], np.float32)
    gate_b = np.asarray(inputs["gate_b"], np.float32)
    w1 = np.asarray(inputs["w1"], np.float32)
    b1 = np.asarray(inputs["b1"], np.float32)
    w2 = np.asarray(inputs["w2"], np.float32)
    b2 = np.asarray(inputs["b2"], np.float32)
    load_ema = np.asarray(inputs["load_ema"], np.float32)
    k = int(np.asarray(inputs["top_k"]))

    B, S, d = x.shape
    n_tok = B * S
    flat = np.ascontiguousarray(x.reshape(n_tok, d), dtype=np.float32)
    n_experts = gate_w.shape[1]

    # ---- host router (this is the sharding decision) ----
    top_idx, weights, logits = _route(flat, gate_w, gate_b, k)

    idx_by_e = []
    wt_by_e = []
    for e in range(n_experts):
        tok, slot = np.nonzero(top_idx == e)
        idx_by_e.append(tok)
        wt_by_e.append(weights[tok, slot].astype(np.float32))
    counts = np.array([len(ix) for ix in idx_by_e])

    C = max(256, 128 * int(np.ceil(counts.max() / 128)))
    variant = os.environ.get("KERNEL_DTYPE", "f32")
    np_mm = np.float32
    if variant == "bf16":
        import ml_dtypes

        np_mm = ml_dtypes.bfloat16

    key = (C, variant, E_LOC)
    if key not in _PROG_CACHE:
        _PROG_CACHE[key] = _build_program(C, variant)
    nc = _PROG_CACHE[key]

    # ---- build per-core inputs (expert slices + gathered token batches) ----
    assign = [list(range(c * E_LOC, (c + 1) * E_LOC)) for c in range(N_CORES)]
    flatT = np.ascontiguousarray(flat.T)  # [D, N]
    FT = F // 128
    CT = C // 128

    in_maps = []
    for core in range(N_CORES):
        exps = assign[core]
        xT_c = np.zeros((E_LOC, D, C), np_mm)
        wt_c = np.zeros((E_LOC, C), np.float32)
        b1_c = np.zeros((E_LOC, 128, FT), np.float32)
        for s, e in enumerate(exps):
            cnt = counts[e]
            xT_c[s, :, :cnt] = flatT[:, idx_by_e[e]].astype(np_mm)
            wt_c[s, :cnt] = wt_by_e[e]
            b1_c[s] = b1[e].reshape(FT, 128).T
        in_maps.append(
            {
                "xT": xT_c,
                "w1": np.ascontiguousarray(w1[exps].astype(np_mm)),
                "b1v": b1_c,
                "w2": np.ascontiguousarray(w2[exps].astype(np_mm)),
                "wt": np.ascontiguousarray(
                    wt_c.reshape(E_LOC, CT, 128).transpose(0, 2, 1)
                ),
            }
        )

    trace = bool(int(os.environ.get("KERNEL_TRACE", "0")))
    if trace:
        _ensure_ntff_hook()
    res = run_bass_kernel_spmd(nc, in_maps, list(range(N_CORES)), trace=trace)
    LAST_RESULTS = res

    # ---- combine (the "all-to-all combine" done host-side) ----
    out = np.zeros((n_tok, d), np.float64)
    for core in range(N_CORES):
        y_c = res.results[core]["y"]
        for s, e in enumerate(assign[core]):
            cnt = counts[e]
            out[idx_by_e[e]] += y_c[s, :cnt].astype(np.float64)
            if b2[e].any():
                out[idx_by_e[e]] += (
                    wt_by_e[e][:, None].astype(np.float64) * b2[e][None, :]
                )

    # ---- aux loss (host, float64) ----
    probs = np.exp(logits - logits.max(axis=1, keepdims=True))
    probs /= probs.sum(axis=1, keepdims=True)
    frac = counts / n_tok / k
    base_aux = n_experts * np.sum(frac * probs.mean(axis=0))
    le = load_ema.astype(np.float64)
    lp = le / (le.sum() + 1e-8)
    entropy = -np.sum(lp * np.log(lp + 1e-8))
    reg = np.log(float(n_experts)) - entropy
    aux = np.float32(base_aux + 0.001 * reg)

    return out.astype(np.float32).reshape(B, S, d), aux


# revision 19
# speedup vs baseline: 2.8947x; 1.4788x over previous
"""Expert-parallel MoE (CompoundMoELayer) kernel for 8 Trainium2 NeuronCores.

Strategy (per the expert-parallelism sharding hint):
  - Router runs once on the host (it is <0.2% of the FLOPs and is the
    sharding decision itself): logits -> top-4 -> softmax combine weights.
  - The 32 experts are sharded 4-per-core, balanced by routed token count
    (rank-octile round-robin).  For each expert the tokens routed to it are
    gathered into an exact-capacity [D, cap] transposed batch so the device
    only does dense GEMMs.
  - Each core runs, per owned expert:  H_T = W1^T @ Xe^T  (PSUM, +b1, exact
    GELU) ; Y = G @ W2 ; Y *= combine_weight.  Y is returned per expert slot.
  - Host scatter-adds the weighted per-expert outputs back to token order
    (the "combine" of the all-to-all) and adds the (combine @ b2) term.
  - The aux scalar (load-balance loss + entropy regularizer) is a tiny
    reduction over the router logits, computed on host in float64.

All device DMAs are arranged to be contiguous in >=2KB runs: w2/w1 tiles are
pure reshapes of the natural weight layout, and the gathered token batches
are packed partition-major on the host.

The device program is an SPMD Bass/Tile kernel launched on all 8 cores via
run_bass_kernel_spmd; per-core inputs differ only in data (expert slices).
"""

import os
import sys

import numpy as np

for _p in ("/opt/trn_rl_repo", "/root/.axon_site/_ro/trn_rl_repo"):
    if os.path.isdir(_p) and _p not in sys.path:
        sys.path.append(_p)

import concourse.bass as bass
import concourse.mybir as mybir
import concourse.tile as tile
from concourse.bass_utils import run_bass_kernel_spmd
from concourse.vector_clock import ScopedClock

# ----------------------------------------------------------------------------
# This container's walrus build only accepts a single sync-wait per
# instruction; TileContext can attach several (one per DMA queue).  Split
# extra waits onto preceding same-engine nops, and the tail drain onto
# multiple drains.
# ----------------------------------------------------------------------------


def _split_multi_waits(nc):
    for bb in nc.m.functions[0].blocks:
        insts = bb.instructions
        i = 0
        while i < len(insts):
            inst = insts[i]
            si = inst.sync_info
            if si is not None and si.on_wait is not None and len(si.on_wait) > 1:
                waits = list(si.on_wait)
                del si.on_wait[: len(waits) - 1]  # keep only the last wait
                eng = inst.engine
                for w in waits[:-1]:
                    nop = mybir.InstNoOp(
                        name=nc.get_next_instruction_name(), ins=[], outs=[]
                    )
                    nop.engine = eng
                    nop.sync_info = mybir.SyncInfo(on_wait=[w], on_update=[])
                    nc.register_instruction(nop, overwrite=True)
                    insts.insert(i, nop)
                    i += 1
            i += 1


def _drain_and_barrier_split(self, tick_clock, wait_clock):
    nc = self.nc
    drain_inst = nc.sync.drain()
    wait_clock.add_sem_waits(
        drain_inst.ins, ScopedClock({None: tick_clock.global_clock})
    )
    si = drain_inst.ins.sync_info
    if si is not None and si.on_wait is not None and len(si.on_wait) > 1:
        waits = list(si.on_wait)
        del si.on_wait[1:]
        for w in waits[1:]:
            extra = nc.sync.drain()
            esi = extra.ins.sync_info
            if esi is None:
                extra.ins.sync_info = mybir.SyncInfo(on_wait=[w], on_update=[])
            else:
                if esi.on_wait is None:
                    esi.on_wait = []
                esi.on_wait.append(w)

    _split_multi_waits(nc)

    nc.all_engine_barrier()
    assert self.sems is not None
    popped = nc._tile_sem_poison_stack.pop()
    assert popped is self._sem_poison
    nc.clear_and_free_semaphores(list(self.sems.allocated().values()))
    nc.all_engine_barrier()


tile.TileContext._drain_and_barrier = _drain_and_barrier_split

# ----------------------------------------------------------------------------
# Problem constants (hardcoded per the harness contract).
# ----------------------------------------------------------------------------
N_CORES = 8
E = 32
D = 1024
F = 2048
E_LOC = E // N_CORES  # experts per core

KT = D // 128  # GEMM1 contraction tiles
FT = F // 128  # FFN tiles (partition dim of H_T)
JT = D // 512  # GEMM2 output-column tiles

_F32 = mybir.dt.float32

_PROG_CACHE: dict = {}
LAST_RESULTS = None  # BassKernelResults of the most recent launch (for test.py)


def _ensure_ntff_hook():
    """Profiling support: this agent image ships no ``antenv.axon_hooks``,
    so ``run_bass_kernel_spmd(trace=True)`` under axon degrades to no
    timing.  Provide the module + the ctypes NTFF hook (mirror of
    trn_boot._ntff_profile_via_ctypes), and neuter the fish upload."""
    import contextlib
    import ctypes
    import types

    name = "antenv.axon_hooks"
    if name not in sys.modules:
        mod = types.ModuleType(name)
        mod._hook = None

        def set_axon_ntff_profile_hook(h):
            mod._hook = h

        def get_axon_ntff_profile_hook():
            return mod._hook

        mod.set_axon_ntff_profile_hook = set_axon_ntff_profile_hook
        mod.get_axon_ntff_profile_hook = get_axon_ntff_profile_hook
        sys.modules[name] = mod

        so_path = "/opt/axon/libaxon_pjrt.so"
        try:
            lib = ctypes.CDLL(so_path)
            lib.axon_start_nrt_profile.argtypes = [
                ctypes.POINTER(ctypes.c_int64),
                ctypes.c_size_t,
            ]
            lib.axon_start_nrt_profile.restype = ctypes.c_int64
            lib.axon_stop_nrt_profile.argtypes = [ctypes.c_char_p]
            lib.axon_stop_nrt_profile.restype = ctypes.c_int64

            @contextlib.contextmanager
            def _hook(output_dir, device_ids):
                import jax

                jax.devices()
                if device_ids:
                    ids = (ctypes.c_int64 * len(device_ids))(*device_ids)
                    rc = lib.axon_start_nrt_profile(ids, len(device_ids))
                else:
                    rc = lib.axon_start_nrt_profile(None, 0)
                if rc != 0:
                    raise RuntimeError(f"axon_start_nrt_profile rc={rc}")
                try:
                    yield
                finally:
                    n = lib.axon_stop_nrt_profile(str(output_dir).encode())
                    print(f"profile: {n} file(s) written to {output_dir}")

            mod.set_axon_ntff_profile_hook(_hook)
        except (OSError, AttributeError):
            pass

    import concourse.bass_utils as _bu

    _bu.upload_artifacts = lambda tmpdir: str(tmpdir)


def _build_program(caps: tuple, variant: str):
    """Per-core SPMD program: E_LOC expert slots with capacities ``caps``.

    variant: "f32" (exact, quarter-rate PE), "f32r" (fp32 storage,
    full-rate single-pass PE, needs moving dim >=256), "bf16".
    """
    dt_mm = {
        "bf16": mybir.dt.bfloat16,
        "f32r": mybir.dt.float32r,
        "f32": _F32,
    }[variant]

    nc = bass.Bass()
    # weights in natural layout (tiles are pure reshapes -> contiguous DMAs)
    w1 = nc.dram_tensor("w1", [E_LOC, KT, 128, F], dt_mm, kind="ExternalInput")
    w2 = nc.dram_tensor("w2", [E_LOC, FT, 128, D], dt_mm, kind="ExternalInput")
    b1v = nc.dram_tensor("b1v", [E_LOC, 128, FT], _F32, kind="ExternalInput")
    xTs, wts, ys = [], [], []
    for s, cap in enumerate(caps):
        ct = (cap + 127) // 128
        xTs.append(
            nc.dram_tensor(f"xT{s}", [128, KT * cap], dt_mm, kind="ExternalInput")
        )
        wts.append(nc.dram_tensor(f"wt{s}", [128, ct], _F32, kind="ExternalInput"))
        ys.append(nc.dram_tensor(f"y{s}", [cap, D], _F32, kind="ExternalOutput"))

    gelu = mybir.ActivationFunctionType.Gelu
    max_cap = max(caps)

    with tile.TileContext(nc) as tc:
        with (
            tc.tile_pool(name="xp", bufs=2) as xp,
            tc.tile_pool(name="w1p", bufs=2) as w1p,
            tc.tile_pool(name="gp", bufs=2) as gp,
            tc.tile_pool(name="w2p", bufs=3) as w2p,
            tc.tile_pool(name="yp", bufs=4) as yp,
            tc.tile_pool(name="cst", bufs=2) as cst,
            tc.tile_pool(name="ps1", bufs=2, space="PSUM") as ps1,
            tc.tile_pool(name="ps2", bufs=1, space="PSUM") as ps2,
        ):
            for s, cap in enumerate(caps):
                CT = (cap + 127) // 128
                mt = [min(128, cap - 128 * c) for c in range(CT)]

                # gathered tokens, packed [partition, (k-tile, token)]
                xsb = xp.tile([128, KT * max_cap], dt_mm, tag="xsb")
                nc.sync.dma_start(xsb[:, : KT * cap], xTs[s][:])
                b1sb = cst.tile([128, FT], _F32, tag="b1sb")
                nc.sync.dma_start(b1sb[:], b1v[s])
                wtsb = cst.tile([128, (max_cap + 127) // 128], _F32, tag="wtsb")
                nc.sync.dma_start(wtsb[:, :CT], wts[s][:])

                # W1 for this expert, fully resident: KT blocks [128, F]
                w1sb = w1p.tile([128, KT, F], dt_mm, tag="w1sb")
                for kk in range(KT):
                    nc.sync.dma_start(w1sb[:, kk, :], w1[s, kk])

                # G^T = gelu(W1^T @ Xe^T + b1), laid out [FFN-part, token]
                gsb = gp.tile([128, FT, max_cap], dt_mm, tag="gsb")
                for f in range(FT):
                    ph = ps1.tile([128, max_cap], _F32, tag="ph")
                    for kk in range(KT):
                        nc.tensor.matmul(
                            ph[:, :cap],
                            w1sb[:, kk, f * 128 : (f + 1) * 128],
                            xsb[:, kk * cap : (kk + 1) * cap],
                            start=(kk == 0),
                            stop=(kk == KT - 1),
                        )
                    nc.scalar.activation(
                        gsb[:, f, :cap],
                        ph[:, :cap],
                        gelu,
                        bias=b1sb[:, f : f + 1],
                    )

                # Y = (G @ W2) * combine_weight; w2 streamed once per fk
                phys = [
                    [
                        ps2.tile([128, 512], _F32, tag=f"py{j}{c}", name=f"py{j}{c}")
                        for c in range(CT)
                    ]
                    for j in range(JT)
                ]
                for fk in range(FT):
                    w2sb = w2p.tile([128, D], dt_mm, tag="w2sb")
                    nc.sync.dma_start(w2sb[:], w2[s, fk])
                    for j in range(JT):
                        for c in range(CT):
                            nc.tensor.matmul(
                                phys[j][c][: mt[c], :],
                                gsb[:, fk, c * 128 : c * 128 + mt[c]],
                                w2sb[:, j * 512 : (j + 1) * 512],
                                start=(fk == 0),
                                stop=(fk == FT - 1),
                            )
                for j in range(JT):
                    for c in range(CT):
                        ysb = yp.tile([128, 512], _F32, tag="ysb")
                        nc.vector.tensor_scalar_mul(
                            ysb[: mt[c], :], phys[j][c][: mt[c], :], wtsb[: mt[c], c : c + 1]
                        )
                        nc.sync.dma_start(
                            ys[s][c * 128 : c * 128 + mt[c], j * 512 : (j + 1) * 512],
                            ysb[: mt[c], :],
                        )
    return nc


def _route(flat, gate_w, gate_b, k):
    """Host router in float64: returns (top_idx [N,k], weights [N,k], logits)."""
    logits = flat.astype(np.float64) @ gate_w.astype(np.float64) + gate_b.astype(
        np.float64
    )
    # stable argsort matches jax.lax.top_k tie-breaking (lower index first)
    top_idx = np.argsort(-logits, axis=1, kind="stable")[:, :k]
    top_vals = np.take_along_axis(logits, top_idx, axis=1)
    w = np.exp(top_vals - top_vals.max(axis=1, keepdims=True))
    w /= w.sum(axis=1, keepdims=True)
    return top_idx, w, logits


def kernel(**inputs) -> tuple:
    global LAST_RESULTS
    x = np.asarray(inputs["x"])
    gate_w = np.asarray(inputs["gate_w"], np.float32)
    gate_b = np.asarray(inputs["gate_b"], np.float32)
    w1 = np.asarray(inputs["w1"], np.float32)
    b1 = np.asarray(inputs["b1"], np.float32)
    w2 = np.asarray(inputs["w2"], np.float32)
    b2 = np.asarray(inputs["b2"], np.float32)
    load_ema = np.asarray(inputs["load_ema"], np.float32)
    k = int(np.asarray(inputs["top_k"]))

    B, S, d = x.shape
    n_tok = B * S
    flat = np.ascontiguousarray(x.reshape(n_tok, d), dtype=np.float32)
    n_experts = gate_w.shape[1]

    # ---- host router (this is the sharding decision) ----
    top_idx, weights, logits = _route(flat, gate_w, gate_b, k)

    idx_by_e = []
    wt_by_e = []
    for e in range(n_experts):
        tok, slot = np.nonzero(top_idx == e)
        idx_by_e.append(tok)
        wt_by_e.append(weights[tok, slot].astype(np.float32))
    counts = np.array([len(ix) for ix in idx_by_e])

    variant = os.environ.get("KERNEL_DTYPE", "bf16")
    np_mm = np.float32
    if variant == "bf16":
        import ml_dtypes

        np_mm = ml_dtypes.bfloat16

    # ---- balanced assignment: rank-octile round-robin; exact slot caps ----
    order = np.argsort(-counts, kind="stable")
    # slot s of core c owns expert order[N_CORES*s + c]
    min_cap = 256 if variant == "f32r" else 16
    caps = []
    for s in range(E_LOC):
        group = counts[order[N_CORES * s : N_CORES * (s + 1)]]
        caps.append(max(min_cap, int(-8 * (-group.max() // 8))))
    caps = tuple(caps)

    key = (caps, variant)
    if key not in _PROG_CACHE:
        _PROG_CACHE[key] = _build_program(caps, variant)
    nc = _PROG_CACHE[key]

    # ---- build per-core inputs (expert slices + gathered token batches) ----
    flatT = np.ascontiguousarray(flat.T)  # [D, N]
    w1_t = w1.reshape(n_experts, KT, 128, F).astype(np_mm)
    w2_t = w2.reshape(n_experts, FT, 128, D).astype(np_mm)

    in_maps = []
    assign = [[int(order[N_CORES * s + c]) for s in range(E_LOC)] for c in range(N_CORES)]
    for core in range(N_CORES):
        exps = assign[core]
        m = {
            "w1": np.ascontiguousarray(w1_t[exps]),
            "w2": np.ascontiguousarray(w2_t[exps]),
            "b1v": np.ascontiguousarray(
                b1[exps].reshape(E_LOC, FT, 128).transpose(0, 2, 1)
            ),
        }
        for s, e in enumerate(exps):
            cap = caps[s]
            cnt = counts[e]
            ct = (cap + 127) // 128
            xslab = np.zeros((KT, 128, cap), np_mm)
            xslab[:, :, :cnt] = (
                flatT[:, idx_by_e[e]].astype(np_mm).reshape(KT, 128, cnt)
            )
            m[f"xT{s}"] = np.ascontiguousarray(
                xslab.transpose(1, 0, 2).reshape(128, KT * cap)
            )
            wt_pad = np.zeros((ct * 128,), np.float32)
            wt_pad[:cnt] = wt_by_e[e]
            m[f"wt{s}"] = np.ascontiguousarray(wt_pad.reshape(ct, 128).T)
        in_maps.append(m)

    trace = bool(int(os.environ.get("KERNEL_TRACE", "0")))
    if trace:
        _ensure_ntff_hook()
    res = run_bass_kernel_spmd(nc, in_maps, list(range(N_CORES)), trace=trace)
    LAST_RESULTS = res

    # ---- combine (the "all-to-all combine" done host-side) ----
    out = np.zeros((n_tok, d), np.float64)
    for core in range(N_CORES):
        rc = res.results[core]
        for s, e in enumerate(assign[core]):
            cnt = counts[e]
            out[idx_by_e[e]] += rc[f"y{s}"][:cnt].astype(np.float64)
            if b2[e].any():
                out[idx_by_e[e]] += (
                    wt_by_e[e][:, None].astype(np.float64) * b2[e][None, :]
                )

    # ---- aux loss (host, float64) ----
    probs = np.exp(logits - logits.max(axis=1, keepdims=True))
    probs /= probs.sum(axis=1, keepdims=True)
    frac = counts / n_tok / k
    base_aux = n_experts * np.sum(frac * probs.mean(axis=0))
    le = load_ema.astype(np.float64)
    lp = le / (le.sum() + 1e-8)
    entropy = -np.sum(lp * np.log(lp + 1e-8))
    reg = np.log(float(n_experts)) - entropy
    aux = np.float32(base_aux + 0.001 * reg)

    return out.astype(np.float32).reshape(B, S, d), aux


# revision 24
# speedup vs baseline: 3.4437x; 1.1897x over previous
"""Expert-parallel MoE (CompoundMoELayer) kernel for 8 Trainium2 NeuronCores.

Strategy (per the expert-parallelism sharding hint):
  - Router runs once on the host (it is <0.2% of the FLOPs and is the
    sharding decision itself): logits -> top-4 -> softmax combine weights.
  - The 32 experts are sharded 4-per-core, balanced by routed token count
    (rank-octile round-robin).  For each expert the tokens routed to it are
    gathered into an exact-capacity [D, cap] transposed batch so the device
    only does dense GEMMs.
  - Each core runs, per owned expert:  H_T = W1^T @ Xe^T  (PSUM, +b1, exact
    GELU) ; Y = G @ W2 ; Y *= combine_weight.  Y is returned per expert slot.
  - Host scatter-adds the weighted per-expert outputs back to token order
    (the "combine" of the all-to-all) and adds the (combine @ b2) term.
  - The aux scalar (load-balance loss + entropy regularizer) is a tiny
    reduction over the router logits, computed on host in float64.

All device DMAs are arranged to be contiguous in >=2KB runs: w2/w1 tiles are
pure reshapes of the natural weight layout, and the gathered token batches
are packed partition-major on the host.

The device program is an SPMD Bass/Tile kernel launched on all 8 cores via
run_bass_kernel_spmd; per-core inputs differ only in data (expert slices).
"""

import os
import sys

import numpy as np

for _p in ("/opt/trn_rl_repo", "/root/.axon_site/_ro/trn_rl_repo"):
    if os.path.isdir(_p) and _p not in sys.path:
        sys.path.append(_p)

import concourse.bass as bass
import concourse.mybir as mybir
import concourse.tile as tile
from concourse.bass_utils import run_bass_kernel_spmd
from concourse.vector_clock import ScopedClock

# ----------------------------------------------------------------------------
# This container's walrus build only accepts a single sync-wait per
# instruction; TileContext can attach several (one per DMA queue).  Split
# extra waits onto preceding same-engine nops, and the tail drain onto
# multiple drains.
# ----------------------------------------------------------------------------


def _split_multi_waits(nc):
    for bb in nc.m.functions[0].blocks:
        insts = bb.instructions
        i = 0
        while i < len(insts):
            inst = insts[i]
            si = inst.sync_info
            if si is not None and si.on_wait is not None and len(si.on_wait) > 1:
                waits = list(si.on_wait)
                del si.on_wait[: len(waits) - 1]  # keep only the last wait
                eng = inst.engine
                for w in waits[:-1]:
                    nop = mybir.InstNoOp(
                        name=nc.get_next_instruction_name(), ins=[], outs=[]
                    )
                    nop.engine = eng
                    nop.sync_info = mybir.SyncInfo(on_wait=[w], on_update=[])
                    nc.register_instruction(nop, overwrite=True)
                    insts.insert(i, nop)
                    i += 1
            i += 1


def _drain_and_barrier_split(self, tick_clock, wait_clock):
    nc = self.nc
    drain_inst = nc.sync.drain()
    wait_clock.add_sem_waits(
        drain_inst.ins, ScopedClock({None: tick_clock.global_clock})
    )
    si = drain_inst.ins.sync_info
    if si is not None and si.on_wait is not None and len(si.on_wait) > 1:
        waits = list(si.on_wait)
        del si.on_wait[1:]
        for w in waits[1:]:
            extra = nc.sync.drain()
            esi = extra.ins.sync_info
            if esi is None:
                extra.ins.sync_info = mybir.SyncInfo(on_wait=[w], on_update=[])
            else:
                if esi.on_wait is None:
                    esi.on_wait = []
                esi.on_wait.append(w)

    _split_multi_waits(nc)

    nc.all_engine_barrier()
    assert self.sems is not None
    popped = nc._tile_sem_poison_stack.pop()
    assert popped is self._sem_poison
    nc.clear_and_free_semaphores(list(self.sems.allocated().values()))
    nc.all_engine_barrier()


tile.TileContext._drain_and_barrier = _drain_and_barrier_split

# ----------------------------------------------------------------------------
# Problem constants (hardcoded per the harness contract).
# ----------------------------------------------------------------------------
N_CORES = 8
E = 32
D = 1024
F = 2048
E_LOC = E // N_CORES  # experts per core

KT = D // 128  # GEMM1 contraction tiles
FT = F // 128  # FFN tiles (partition dim of H_T)
JT = D // 512  # GEMM2 output-column tiles

_F32 = mybir.dt.float32

_PROG_CACHE: dict = {}
LAST_RESULTS = None  # BassKernelResults of the most recent launch (for test.py)


def _ensure_ntff_hook():
    """Profiling support: this agent image ships no ``antenv.axon_hooks``,
    so ``run_bass_kernel_spmd(trace=True)`` under axon degrades to no
    timing.  Provide the module + the ctypes NTFF hook (mirror of
    trn_boot._ntff_profile_via_ctypes), and neuter the fish upload."""
    import contextlib
    import ctypes
    import types

    name = "antenv.axon_hooks"
    if name not in sys.modules:
        mod = types.ModuleType(name)
        mod._hook = None

        def set_axon_ntff_profile_hook(h):
            mod._hook = h

        def get_axon_ntff_profile_hook():
            return mod._hook

        mod.set_axon_ntff_profile_hook = set_axon_ntff_profile_hook
        mod.get_axon_ntff_profile_hook = get_axon_ntff_profile_hook
        sys.modules[name] = mod

        so_path = "/opt/axon/libaxon_pjrt.so"
        try:
            lib = ctypes.CDLL(so_path)
            lib.axon_start_nrt_profile.argtypes = [
                ctypes.POINTER(ctypes.c_int64),
                ctypes.c_size_t,
            ]
            lib.axon_start_nrt_profile.restype = ctypes.c_int64
            lib.axon_stop_nrt_profile.argtypes = [ctypes.c_char_p]
            lib.axon_stop_nrt_profile.restype = ctypes.c_int64

            @contextlib.contextmanager
            def _hook(output_dir, device_ids):
                import jax

                jax.devices()
                if device_ids:
                    ids = (ctypes.c_int64 * len(device_ids))(*device_ids)
                    rc = lib.axon_start_nrt_profile(ids, len(device_ids))
                else:
                    rc = lib.axon_start_nrt_profile(None, 0)
                if rc != 0:
                    raise RuntimeError(f"axon_start_nrt_profile rc={rc}")
                try:
                    yield
                finally:
                    n = lib.axon_stop_nrt_profile(str(output_dir).encode())
                    print(f"profile: {n} file(s) written to {output_dir}")

            mod.set_axon_ntff_profile_hook(_hook)
        except (OSError, AttributeError):
            pass

    import concourse.bass_utils as _bu

    _bu.upload_artifacts = lambda tmpdir: str(tmpdir)


def _build_program(caps: tuple, variant: str):
    """Per-core SPMD program: E_LOC expert slots with capacities ``caps``.

    variant: "f32" (exact, quarter-rate PE), "f32r" (fp32 storage,
    full-rate single-pass PE, needs moving dim >=256), "bf16".
    """
    dt_mm = {
        "bf16": mybir.dt.bfloat16,
        "f32r": mybir.dt.float32r,
        "f32": _F32,
    }[variant]

    nc = bass.Bass()
    # weights in natural layout (tiles are pure reshapes -> contiguous DMAs)
    w1 = nc.dram_tensor("w1", [E_LOC, D, F], dt_mm, kind="ExternalInput")
    w2 = nc.dram_tensor("w2", [E_LOC, F, D], dt_mm, kind="ExternalInput")
    b1v = nc.dram_tensor("b1v", [E_LOC, 128, FT], _F32, kind="ExternalInput")
    xTs, wts, ys = [], [], []
    for s, cap in enumerate(caps):
        ct = (cap + 127) // 128
        xTs.append(
            nc.dram_tensor(f"xT{s}", [128, KT * cap], dt_mm, kind="ExternalInput")
        )
        wts.append(nc.dram_tensor(f"wt{s}", [128, ct], _F32, kind="ExternalInput"))
        ys.append(nc.dram_tensor(f"y{s}", [cap, D], _F32, kind="ExternalOutput"))

    gelu = mybir.ActivationFunctionType.Gelu
    max_cap = max(caps)

    with tile.TileContext(nc) as tc:
        with (
            tc.tile_pool(name="xp", bufs=2) as xp,
            tc.tile_pool(name="w1p", bufs=2) as w1p,
            tc.tile_pool(name="gp", bufs=2) as gp,
            tc.tile_pool(name="w2p", bufs=3) as w2p,
            tc.tile_pool(name="yp", bufs=4) as yp,
            tc.tile_pool(name="cst", bufs=2) as cst,
            tc.tile_pool(name="ps1", bufs=3, space="PSUM") as ps1,
            tc.tile_pool(name="ps2", bufs=1, space="PSUM") as ps2,
        ):
            for s, cap in enumerate(caps):
                CT = (cap + 127) // 128
                mt = [min(128, cap - 128 * c) for c in range(CT)]

                # gathered tokens, packed [partition, (k-tile, token)]
                xsb = xp.tile([128, KT * max_cap], dt_mm, tag="xsb")
                nc.sync.dma_start(xsb[:, : KT * cap], xTs[s][:])
                b1sb = cst.tile([128, FT], _F32, tag="b1sb")
                nc.sync.dma_start(b1sb[:], b1v[s])
                wtsb = cst.tile([128, (max_cap + 127) // 128], _F32, tag="wtsb")
                nc.sync.dma_start(wtsb[:, :CT], wts[s][:])

                # W1 for this expert, fully resident (one DMA, 4KB runs)
                w1sb = w1p.tile([128, KT, F], dt_mm, tag="w1sb")
                nc.sync.dma_start(
                    w1sb[:], w1[s].rearrange("(kk p) f -> p kk f", p=128)
                )

                # G^T = gelu(W1^T @ Xe^T + b1), laid out [FFN-part, token]
                gsb = gp.tile([128, FT, max_cap], dt_mm, tag="gsb")
                for f in range(FT):
                    ph = ps1.tile([128, max_cap], _F32, tag="ph")
                    for kk in range(KT):
                        nc.tensor.matmul(
                            ph[:, :cap],
                            w1sb[:, kk, f * 128 : (f + 1) * 128],
                            xsb[:, kk * cap : (kk + 1) * cap],
                            start=(kk == 0),
                            stop=(kk == KT - 1),
                        )
                    nc.scalar.activation(
                        gsb[:, f, :cap],
                        ph[:, :cap],
                        gelu,
                        bias=b1sb[:, f : f + 1],
                    )

                # Y = (G @ W2) * combine_weight; w2 streamed once per fk
                phys = [
                    [
                        ps2.tile([128, 512], _F32, tag=f"py{j}{c}", name=f"py{j}{c}")
                        for c in range(CT)
                    ]
                    for j in range(JT)
                ]
                for fg in range(FT // 2):
                    w2sb = w2p.tile([128, 2, D], dt_mm, tag="w2sb")
                    nc.sync.dma_start(
                        w2sb[:],
                        w2[s, fg * 256 : (fg + 1) * 256, :].rearrange(
                            "(g p) d -> p g d", p=128
                        ),
                    )
                    for g in range(2):
                        fk = fg * 2 + g
                        for j in range(JT):
                            for c in range(CT):
                                nc.tensor.matmul(
                                    phys[j][c][: mt[c], :],
                                    gsb[:, fk, c * 128 : c * 128 + mt[c]],
                                    w2sb[:, g, j * 512 : (j + 1) * 512],
                                    start=(fk == 0),
                                    stop=(fk == FT - 1),
                                )
                for j in range(JT):
                    for c in range(CT):
                        ysb = yp.tile([128, 512], _F32, tag="ysb")
                        nc.vector.tensor_scalar_mul(
                            ysb[: mt[c], :], phys[j][c][: mt[c], :], wtsb[: mt[c], c : c + 1]
                        )
                        nc.sync.dma_start(
                            ys[s][c * 128 : c * 128 + mt[c], j * 512 : (j + 1) * 512],
                            ysb[: mt[c], :],
                        )
    return nc


def _route(flat, gate_w, gate_b, k):
    """Host router in float64: returns (top_idx [N,k], weights [N,k], logits)."""
    logits = flat.astype(np.float64) @ gate_w.astype(np.float64) + gate_b.astype(
        np.float64
    )
    # stable argsort matches jax.lax.top_k tie-breaking (lower index first)
    top_idx = np.argsort(-logits, axis=1, kind="stable")[:, :k]
    top_vals = np.take_along_axis(logits, top_idx, axis=1)
    w = np.exp(top_vals - top_vals.max(axis=1, keepdims=True))
    w /= w.sum(axis=1, keepdims=True)
    return top_idx, w, logits


def kernel(**inputs) -> tuple:
    global LAST_RESULTS
    x = np.asarray(inputs["x"])
    gate_w = np.asarray(inputs["gate_w"], np.float32)
    gate_b = np.asarray(inputs["gate_b"], np.float32)
    w1 = np.asarray(inputs["w1"], np.float32)
    b1 = np.asarray(inputs["b1"], np.float32)
    w2 = np.asarray(inputs["w2"], np.float32)
    b2 = np.asarray(inputs["b2"], np.float32)
    load_ema = np.asarray(inputs["load_ema"], np.float32)
    k = int(np.asarray(inputs["top_k"]))

    B, S, d = x.shape
    n_tok = B * S
    flat = np.ascontiguousarray(x.reshape(n_tok, d), dtype=np.float32)
    n_experts = gate_w.shape[1]

    # ---- host router (this is the sharding decision) ----
    top_idx, weights, logits = _route(flat, gate_w, gate_b, k)

    idx_by_e = []
    wt_by_e = []
    for e in range(n_experts):
        tok, slot = np.nonzero(top_idx == e)
        idx_by_e.append(tok)
        wt_by_e.append(weights[tok, slot].astype(np.float32))
    counts = np.array([len(ix) for ix in idx_by_e])

    variant = os.environ.get("KERNEL_DTYPE", "bf16")
    np_mm = np.float32
    if variant == "bf16":
        import ml_dtypes

        np_mm = ml_dtypes.bfloat16

    # ---- balanced assignment: rank-octile round-robin; exact slot caps ----
    order = np.argsort(-counts, kind="stable")
    # slot s of core c owns expert order[N_CORES*s + c]
    min_cap = 256 if variant == "f32r" else 16
    caps = []
    for s in range(E_LOC):
        group = counts[order[N_CORES * s : N_CORES * (s + 1)]]
        caps.append(max(min_cap, int(-8 * (-group.max() // 8))))
    caps = tuple(caps)

    key = (caps, variant)
    if key not in _PROG_CACHE:
        _PROG_CACHE[key] = _build_program(caps, variant)
    nc = _PROG_CACHE[key]

    # ---- build per-core inputs (expert slices + gathered token batches) ----
    flatT = np.ascontiguousarray(flat.T)  # [D, N]
    w1_t = w1.astype(np_mm)
    w2_t = w2.astype(np_mm)

    in_maps = []
    assign = [[int(order[N_CORES * s + c]) for s in range(E_LOC)] for c in range(N_CORES)]
    for core in range(N_CORES):
        exps = assign[core]
        m = {
            "w1": np.ascontiguousarray(w1_t[exps]),
            "w2": np.ascontiguousarray(w2_t[exps]),
            "b1v": np.ascontiguousarray(
                b1[exps].reshape(E_LOC, FT, 128).transpose(0, 2, 1)
            ),
        }
        for s, e in enumerate(exps):
            cap = caps[s]
            cnt = counts[e]
            ct = (cap + 127) // 128
            xslab = np.zeros((KT, 128, cap), np_mm)
            xslab[:, :, :cnt] = (
                flatT[:, idx_by_e[e]].astype(np_mm).reshape(KT, 128, cnt)
            )
            m[f"xT{s}"] = np.ascontiguousarray(
                xslab.transpose(1, 0, 2).reshape(128, KT * cap)
            )
            wt_pad = np.zeros((ct * 128,), np.float32)
            wt_pad[:cnt] = wt_by_e[e]
            m[f"wt{s}"] = np.ascontiguousarray(wt_pad.reshape(ct, 128).T)
        in_maps.append(m)

    trace = bool(int(os.environ.get("KERNEL_TRACE", "0")))
    if trace:
        _ensure_ntff_hook()
    res = run_bass_kernel_spmd(nc, in_maps, list(range(N_CORES)), trace=trace)
    LAST_RESULTS = res

    # ---- combine (the "all-to-all combine" done host-side) ----
    out = np.zeros((n_tok, d), np.float64)
    for core in range(N_CORES):
        rc = res.results[core]
        for s, e in enumerate(assign[core]):
            cnt = counts[e]
            out[idx_by_e[e]] += rc[f"y{s}"][:cnt].astype(np.float64)
            if b2[e].any():
                out[idx_by_e[e]] += (
                    wt_by_e[e][:, None].astype(np.float64) * b2[e][None, :]
                )

    # ---- aux loss (host, float64) ----
    probs = np.exp(logits - logits.max(axis=1, keepdims=True))
    probs /= probs.sum(axis=1, keepdims=True)
    frac = counts / n_tok / k
    base_aux = n_experts * np.sum(frac * probs.mean(axis=0))
    le = load_ema.astype(np.float64)
    lp = le / (le.sum() + 1e-8)
    entropy = -np.sum(lp * np.log(lp + 1e-8))
    reg = np.log(float(n_experts)) - entropy
    aux = np.float32(base_aux + 0.001 * reg)

    return out.astype(np.float32).reshape(B, S, d), aux
